# revision 3
# baseline (speedup 1.0000x reference)
"""Multi-head causal attention kernel for Trainium2 (8 NeuronCores, batch-parallel).

Problem: B=8, Tx=Tz=1024, Dx=Dz=1024, Datt=Dmid=64, H=16, Dout=1024, causal mask.
Sharding: batch dim across the 8 cores (one batch element per core) - weights
replicated, no collectives needed.

Per-core dataflow (all matmul accumulation in fp32 PSUM):
  warmup: ~16 junk matmuls on a memset tile at t=0 so the PE HAM clock-gate is
          warm (2.4 GHz) by the time real data arrives via DMA
  zT    : host-transposed z activations [d, t] fp16 (V path)
  x8/z8 : host-transposed activations [d, t] fp8-e4m3 (Q/K path)
  V     = zT.T @ Wv_cat + bv    -> [z, h*65+e] with a ones column per head
                                   (bias via K=1 matmul; ones col via DVE fill)
  per head-pair P (2 heads at partition offsets 0/64):
    QT_P = Wq8[:,P].T @ x8T + bq  -> [128(he), 1024(x)]  fp8 DoubleRow matmuls
    KT_P = Wk8[:,P].T @ z8T + bk  -> [128(he), 1024(z)]  (2 k-tiles per MM, 1.7x)
    S^T  = lhsT=KT[64,128] x rhs=QT[64,512] -> 2-bank psum [z, 2*x] (row-packed,
           both heads concurrent via row-group tiling)
    A^T  = exp(S^T/8) both heads in one ACT op, causal-trimmed, diag masked (DVE)
    yT   = V_aug.T @ A^T -> psum [65, x]: rows 0..63 = y^T, row 64 = sumexp
    norm: 1/sumexp directly from psum row (DVE) -> partition_broadcast (GPSIMD)
          -> mul (DVE)
  out  = yT_cat.T @ Wp + bp  (psum -> SBUF via DVE -> DRAM)
"""
import sys
import types

sys.path.insert(0, "/opt/trn_rl_repo")

# bass_utils imports antenv.axon_hooks when tracing is requested (e.g. via a
# BASS_TRACE env var); that module doesn't exist in this image. Provide a
# no-op stub so tracing degrades gracefully instead of crashing. A test
# harness can pre-register a real hook module before importing this file.
if "antenv.axon_hooks" not in sys.modules:
    _m = types.ModuleType("antenv.axon_hooks")
    _m.get_axon_ntff_profile_hook = lambda: None
    sys.modules["antenv.axon_hooks"] = _m

import ml_dtypes
import numpy as np

import concourse.bacc as bacc
import concourse.mybir as mybir
import concourse.tile as tile
from concourse.bass_utils import run_bass_kernel_spmd

F32 = mybir.dt.float32
FP16 = mybir.dt.float16
FP8 = mybir.dt.float8e4
E4M3 = ml_dtypes.float8_e4m3

B, T, D, E, H = 8, 1024, 1024, 64, 16
NK = D // 128          # 8 contraction tiles
NKP = NK // 2          # 4 fp8 DoubleRow contraction pairs
NP = H // 2            # 8 head pairs
NJ = T // 128          # 8 z tiles
NC = T // 512          # 2 x chunks
SCALE = 0.125          # 1/sqrt(64)


def build_program():
    nc = bacc.Bacc("TRN2", target_bir_lowering=False, debug=False)

    zT_d = nc.dram_tensor("zT", [D, T], FP16, kind="ExternalInput")
    x8_d = nc.dram_tensor("x8", [D, T], FP8, kind="ExternalInput")
    z8_d = nc.dram_tensor("z8", [D, T], FP8, kind="ExternalInput")
    wq_d = nc.dram_tensor("wq", [D, H * E], FP8, kind="ExternalInput")
    wk_d = nc.dram_tensor("wk", [D, H * E], FP8, kind="ExternalInput")
    wv_d = nc.dram_tensor("wv", [D, H * E], FP16, kind="ExternalInput")
    wp_d = nc.dram_tensor("wp", [H * E, D], FP16, kind="ExternalInput")
    bqk_d = nc.dram_tensor("bqk", [128, 16], F32, kind="ExternalInput")
    bvb_d = nc.dram_tensor("bvb", [128, H * E], FP16, kind="ExternalInput")
    bpb_d = nc.dram_tensor("bpb", [128, H * E], F32, kind="ExternalInput")
    maskt_d = nc.dram_tensor("maskt", [128, 256], FP16, kind="ExternalInput")
    out_d = nc.dram_tensor("out", [T, D], F32, kind="ExternalOutput")

    Exp = mybir.ActivationFunctionType.Exp
    DR = mybir.MatmulPerfMode.DoubleRow

    with tile.TileContext(nc) as tc:
        with (
            tc.tile_pool(name="big", bufs=1) as big,
            tc.tile_pool(name="wf", bufs=2) as wf,
            tc.tile_pool(name="wb", bufs=4) as wb,
            tc.tile_pool(name="qk", bufs=4) as qk,
            tc.tile_pool(name="apool", bufs=8) as apool,
            tc.tile_pool(name="norm", bufs=3) as norm,
            tc.tile_pool(name="opool", bufs=3) as opool,
            tc.tile_pool(name="cst", bufs=1) as cst,
            tc.tile_pool(name="mps", bufs=2, space="PSUM") as mps,
            tc.tile_pool(name="sps", bufs=2, space="PSUM") as sps,
            tc.tile_pool(name="yps", bufs=2, space="PSUM") as yps,
        ):
            # ---- HAM warmup: junk matmuls with no DMA deps keep the PE busy
            # through its 3.4us activity window so real work runs at 2.4 GHz ----
            warm_t = cst.tile([128, 512], FP16)
            nc.gpsimd.memset(warm_t[:], 0.0)
            wps = mps.tile([128, 512], F32, tag="mps", name="warmps")
            for _ in range(16):
                nc.tensor.matmul(wps[:], warm_t[:, 0:128], warm_t[:],
                                 start=True, stop=True)

            # ---- constants ----
            bqk_t = cst.tile([128, 16], F32)
            bvb_t = cst.tile([128, H * E], FP16)
            bpb_t = cst.tile([128, H * E], F32)
            maskt_t = cst.tile([128, 256], FP16)
            onesf_t = cst.tile([128, 16], FP16)
            def _load_consts():
                nc.sync.dma_start(bqk_t[:], bqk_d.ap())
                nc.sync.dma_start(bvb_t[:], bvb_d.ap())
                nc.sync.dma_start(bpb_t[:], bpb_d.ap())
                nc.sync.dma_start(maskt_t[:], maskt_d.ap())
            nc.gpsimd.memset(onesf_t[:], 1.0)

            # ---- resident activations ----
            zT_t = [big.tile([128, T], FP16, tag="zTk", bufs=NK, name=f"zT{k}")
                    for k in range(NK)]
            x8_t = big.tile([128, NK, T], FP8, tag="x8")
            z8_t = big.tile([128, NK, T], FP8, tag="z8")
            V_t = big.tile([128, NJ, H * 65], FP16, tag="V")
            yT_t = big.tile([128, NP, T], FP16, tag="yT")
            wv_r = wv_d.ap().rearrange("(k p) he -> p k he", p=128)
            wp_r = wp_d.ap().rearrange("(k p) dout -> p k dout", p=128)
            wq_r = wq_d.ap().rearrange("(k p) he -> p k he", p=128)
            wk_r = wk_d.ap().rearrange("(k p) he -> p k he", p=128)
            x8_r = x8_d.ap().rearrange("(k p) t -> p k t", p=128)
            z8_r = z8_d.ap().rearrange("(k p) t -> p k t", p=128)
            zT_r = zT_d.ap().rearrange("(k p) t -> p k t", p=128)
            # zT + first wv half first, per-k tiles so the first V matmul only
            # waits on the k=0 chunks (dep tracking is tile-granular)
            wvh0 = [wf.tile([128, 512], FP16, tag="wv0", bufs=NK, name=f"wvh0_{k}")
                    for k in range(NK)]
            for k in range(NK):
                nc.sync.dma_start(zT_t[k][:], zT_r[:, k, :])
                nc.sync.dma_start(wvh0[k][:], wv_r[:, k, 0:512])
            _load_consts()
            nc.sync.dma_start(z8_t[:], z8_r[:, :, :])
            nc.sync.dma_start(x8_t[:], x8_r[:, :, :])

            # ---- V phase: V[z, he] = zT.T @ Wv + bv (65-col/head layout) ----
            for vc in range(2):
                wvh = None
                if vc == 1:
                    wvh = wf.tile([128, NK, 512], FP16, tag="wf")
                    nc.sync.dma_start(wvh[:], wv_r[:, :, 512:1024])
                for zb in range(NJ):
                    ps = mps.tile([128, 512], F32, tag="mps")
                    for k in range(NK):
                        rhs = wvh0[k][:] if vc == 0 else wvh[:, k, :]
                        nc.tensor.matmul(
                            ps[:], zT_t[k][:, zb * 128:(zb + 1) * 128], rhs,
                            start=(k == 0), stop=(k == NK - 1),
                        )
                    dst = V_t[:, zb, vc * 520:(vc + 1) * 520].rearrange(
                        "p (h c) -> p h c", c=65)[:, :, 0:64]
                    nc.vector.tensor_add(
                        dst, ps[:].rearrange("p (h c) -> p h c", c=64),
                        bvb_t[:, vc * 512:(vc + 1) * 512].rearrange("p (h c) -> p h c", c=64))
            for zb in range(NJ):
                ones_dst = V_t[:, zb, :].rearrange("p (h c) -> p h c", c=65)[:, :, 64:65]
                nc.vector.tensor_copy(ones_dst, onesf_t[:].rearrange("p (h c) -> p h c", c=1))

            # ---- prefetch both Wp halves (wf slots free up after V-phase use) ----
            wph = []
            for dc in range(2):
                w = wf.tile([128, NK, 512], FP16, tag="wf", name=f"wph{dc}")
                nc.sync.dma_start(w[:], wp_r[:, :, dc * 512:(dc + 1) * 512])
                wph.append(w)

            # ---- head-pair loop ----
            for P in range(NP):
                wqP = wb.tile([128, NK, 128], FP8, tag="wb")
                nc.sync.dma_start(wqP[:], wq_r[:, :, P * 128:(P + 1) * 128])
                wkP = wb.tile([128, NK, 128], FP8, tag="wb")
                nc.sync.dma_start(wkP[:], wk_r[:, :, P * 128:(P + 1) * 128])

                QT = qk.tile([128, T], FP16, tag="qk")
                for c in range(NC):
                    ps = mps.tile([128, 512], F32, tag="mps")
                    for kp in range(NKP):
                        nc.tensor.matmul(
                            ps[:], wqP[:, 2 * kp:2 * kp + 2, :],
                            x8_t[:, 2 * kp:2 * kp + 2, c * 512:(c + 1) * 512],
                            start=(kp == 0), stop=(kp == NKP - 1),
                            perf_mode=DR,
                        )
                    nc.vector.tensor_scalar_add(QT[:, c * 512:(c + 1) * 512], ps[:],
                                                bqk_t[:, P:P + 1])
                KT = qk.tile([128, T], FP16, tag="qk")
                for c in range(NC):
                    ps = mps.tile([128, 512], F32, tag="mps")
                    for kp in range(NKP):
                        nc.tensor.matmul(
                            ps[:], wkP[:, 2 * kp:2 * kp + 2, :],
                            z8_t[:, 2 * kp:2 * kp + 2, c * 512:(c + 1) * 512],
                            start=(kp == 0), stop=(kp == NKP - 1),
                            perf_mode=DR,
                        )
                    nc.vector.tensor_scalar_add(KT[:, c * 512:(c + 1) * 512], ps[:],
                                                bqk_t[:, 8 + P:9 + P])

                # attention for the two heads of this pair
                for c in range(NC):
                    jlive = [j for j in range(NJ) if 128 * j <= 512 * c + 511]
                    yp = [yps.tile([65, 512], F32, tag="yps", name=f"yp{P}_{c}_{h01}")
                          for h01 in range(2)]
                    for j in jlive:
                        kband = j - 4 * c
                        x0 = 128 * max(kband, 0)
                        sp = sps.tile([128, 1024], F32, tag="sps")
                        at = apool.tile([128, 1024], FP16, tag="at")
                        for h01 in range(2):
                            hoff = 64 * h01
                            nc.tensor.matmul(
                                sp[:, h01 * 512 + x0:(h01 + 1) * 512],
                                KT[hoff:hoff + 64, j * 128:(j + 1) * 128],
                                QT[hoff:hoff + 64, c * 512 + x0:(c + 1) * 512],
                                start=True, stop=True,
                            )
                        # one exp over both heads' regions (strided 2-bank AP)
                        sp_v = sp[:].rearrange("p (h x) -> p h x", x=512)[:, :, x0:512]
                        at_v = at[:].rearrange("p (h x) -> p h x", x=512)[:, :, x0:512]
                        nc.scalar.activation(at_v, sp_v, Exp, bias=0.0, scale=SCALE)
                        if kband >= 0:
                            at_m = at[:].rearrange(
                                "p (h x) -> p h x", x=512)[:, :, x0:x0 + 128]
                            mk_m = maskt_t[:].rearrange("p (h x) -> p h x", x=128)
                            nc.vector.tensor_mul(at_m, at_m, mk_m)
                        for h01 in range(2):
                            h = 2 * P + h01
                            nc.tensor.matmul(
                                yp[h01][:, x0:512],
                                V_t[:, j, h * 65:(h + 1) * 65],
                                at[:, h01 * 512 + x0:(h01 + 1) * 512],
                                start=(j == jlive[0]), stop=(j == jlive[-1]),
                                skip_group_check=True,
                            )
                    # normalization + eviction to packed pair layout
                    # (sumexp must round-trip through SBUF: reciprocal_approx_fast
                    # is a bitwise-seed op and PSUM's e10m23 bits are not fp32)
                    for h01 in range(2):
                        hoff = 64 * h01
                        se_t = norm.tile([1, 512], F32, tag="se")
                        nc.vector.tensor_copy(se_t[:], yp[h01][64:65, :])
                        r_t = norm.tile([1, 512], F32, tag="rt")
                        nc.vector.reciprocal_approx_fast(r_t[:], se_t[:])
                        bc_t = norm.tile([64, 512], F32, tag="bc")
                        nc.gpsimd.partition_broadcast(bc_t[:], r_t[:])
                        nc.vector.tensor_mul(
                            yT_t[hoff:hoff + 64, P, c * 512:(c + 1) * 512],
                            yp[h01][0:64, :], bc_t[:])

            # ---- output projection: out = yT_cat.T @ Wp + bp ----
            for dc in range(2):
                for m in range(NJ):
                    ps = mps.tile([128, 512], F32, tag="mps")
                    for ht in range(NP):
                        nc.tensor.matmul(
                            ps[:], yT_t[:, ht, m * 128:(m + 1) * 128], wph[dc][:, ht, :],
                            start=(ht == 0), stop=(ht == NP - 1),
                        )
                    o_t = opool.tile([128, 512], F32, tag="ot")
                    nc.vector.tensor_add(o_t[:], ps[:], bpb_t[:, dc * 512:(dc + 1) * 512])
                    nc.sync.dma_start(
                        out_d.ap()[m * 128:(m + 1) * 128, dc * 512:(dc + 1) * 512],
                        o_t[:])

    nc.compile()
    return nc


_CACHED_NC = None


def _get_program():
    global _CACHED_NC
    if _CACHED_NC is None:
        _CACHED_NC = build_program()
    return _CACHED_NC


def _prep_shared(Wq, bq, Wk, bk, Wv, bv, Wp, bp, mask):
    assert np.array_equal(
        np.asarray(mask), np.tril(np.ones((T, T), dtype=bool))
    ), "kernel specialized for causal (tril) mask"
    wq = np.ascontiguousarray(
        np.asarray(Wq, np.float32).transpose(1, 0, 2).reshape(D, H * E).astype(E4M3))
    wk = np.ascontiguousarray(
        np.asarray(Wk, np.float32).transpose(1, 0, 2).reshape(D, H * E).astype(E4M3))
    wv = np.ascontiguousarray(
        np.asarray(Wv, np.float32).transpose(1, 0, 2).reshape(D, H * E).astype(np.float16))
    wp = np.ascontiguousarray(np.asarray(Wp, np.float32).astype(np.float16))
    bq_c = np.asarray(bq, np.float32).reshape(-1)
    bk_c = np.asarray(bk, np.float32).reshape(-1)
    bqk = np.concatenate(
        [bq_c.reshape(8, 128).T, bk_c.reshape(8, 128).T], axis=1
    ).astype(np.float32)
    tri = np.triu(np.ones((128, 128), np.float16))  # allow z <= x
    maskt = np.concatenate([tri, tri], axis=1)      # [128, 256] for both heads
    bvb = np.ascontiguousarray(np.broadcast_to(
        np.asarray(bv, np.float32).reshape(1, -1), (128, H * E)).astype(np.float16))
    bpb = np.ascontiguousarray(np.broadcast_to(
        np.asarray(bp, np.float32).reshape(1, -1), (128, H * E)).astype(np.float32))
    return {
        "wq": wq, "wk": wk, "wv": wv, "wp": wp,
        "bqk": np.ascontiguousarray(bqk),
        "bvb": bvb, "bpb": bpb,
        "maskt": np.ascontiguousarray(maskt),
    }


def kernel(x, z, Wq, bq, Wk, bk, Wv, bv, Wp, bp, mask, _trace=False, _trace_kwargs=None):
    x = np.asarray(x, np.float32)
    z = np.asarray(z, np.float32)
    shared = _prep_shared(Wq, bq, Wk, bk, Wv, bv, Wp, bp, mask)
    in_maps = []
    for b in range(B):
        m = dict(shared)
        zt = np.ascontiguousarray(z[b].T)
        m["zT"] = zt.astype(np.float16)
        m["z8"] = zt.astype(E4M3)
        m["x8"] = np.ascontiguousarray(x[b].T).astype(E4M3)
        in_maps.append(m)
    nc = _get_program()
    res = run_bass_kernel_spmd(
        nc, in_maps, core_ids=list(range(B)),
        trace=_trace, **(_trace_kwargs or {}),
    )
    out = np.stack([r["out"] for r in res.results]).astype(np.float32)
    if _trace:
        kernel.last_results = res
    return out


# revision 9
# speedup vs baseline: 1.0167x; 1.0167x over previous
"""Multi-head causal attention kernel for Trainium2 (8 NeuronCores, batch-parallel).

Problem: B=8, Tx=Tz=1024, Dx=Dz=1024, Datt=Dmid=64, H=16, Dout=1024, causal mask.
Sharding: batch dim across the 8 cores (one batch element per core) - weights
replicated, no collectives needed.

Per-core dataflow (all matmul accumulation in fp32 PSUM):
  warmup: ~16 junk matmuls on a memset tile at t=0 so the PE HAM clock-gate is
          warm (2.4 GHz) by the time real data arrives via DMA
  zT    : host-transposed z activations [d, t] fp16 (V path)
  x8/z8 : host-transposed activations [d, t] fp8-e4m3 (Q/K path)
  V     = zT.T @ Wv_cat + bv    -> [z, h*65+e] with a ones column per head
                                   (bias via K=1 matmul; ones col via DVE fill)
  per head-pair P (2 heads at partition offsets 0/64):
    QT_P = Wq8[:,P].T @ x8T + bq  -> [128(he), 1024(x)]  fp8 DoubleRow matmuls
    KT_P = Wk8[:,P].T @ z8T + bk  -> [128(he), 1024(z)]  (2 k-tiles per MM, 1.7x)
    S^T  = lhsT=KT[64,128] x rhs=QT[64,512] -> 2-bank psum [z, 2*x] (row-packed,
           both heads concurrent via row-group tiling)
    A^T  = exp(S^T/8) both heads in one ACT op, causal-trimmed, diag masked (DVE)
    yT   = V_aug.T @ A^T -> psum [65, x]: rows 0..63 = y^T, row 64 = sumexp
    norm: 1/sumexp directly from psum row (DVE) -> partition_broadcast (GPSIMD)
          -> mul (DVE)
  out  = yT_cat.T @ Wp + bp  (psum -> SBUF via DVE -> DRAM)
"""
import sys
import types

sys.path.insert(0, "/opt/trn_rl_repo")

# bass_utils imports antenv.axon_hooks when tracing is requested (e.g. via a
# BASS_TRACE env var); that module doesn't exist in this image. Provide a
# no-op stub so tracing degrades gracefully instead of crashing. A test
# harness can pre-register a real hook module before importing this file.
if "antenv.axon_hooks" not in sys.modules:
    _m = types.ModuleType("antenv.axon_hooks")
    _m.get_axon_ntff_profile_hook = lambda: None
    sys.modules["antenv.axon_hooks"] = _m

import ml_dtypes
import numpy as np

import concourse.bacc as bacc
import concourse.mybir as mybir
import concourse.tile as tile
from concourse.bass_utils import run_bass_kernel_spmd

F32 = mybir.dt.float32
FP16 = mybir.dt.float16
FP8 = mybir.dt.float8e4
E4M3 = ml_dtypes.float8_e4m3

B, T, D, E, H = 8, 1024, 1024, 64, 16
NK = D // 128          # 8 contraction tiles
NKP = NK // 2          # 4 fp8 DoubleRow contraction pairs
NP = H // 2            # 8 head pairs
NJ = T // 128          # 8 z tiles
NC = T // 512          # 2 x chunks
SCALE = 0.125          # 1/sqrt(64)


def build_program():
    nc = bacc.Bacc("TRN2", target_bir_lowering=False, debug=False)

    zT_d = nc.dram_tensor("zT", [D, T], FP16, kind="ExternalInput")
    x8_d = nc.dram_tensor("x8", [D, T], FP8, kind="ExternalInput")
    z8_d = nc.dram_tensor("z8", [D, T], FP8, kind="ExternalInput")
    wq_d = nc.dram_tensor("wq", [D, H * E], FP8, kind="ExternalInput")
    wk_d = nc.dram_tensor("wk", [D, H * E], FP8, kind="ExternalInput")
    wv_d = nc.dram_tensor("wv", [D, H * E], FP16, kind="ExternalInput")
    wp_d = nc.dram_tensor("wp", [H * E, D], FP16, kind="ExternalInput")
    bqk_d = nc.dram_tensor("bqk", [128, 16], F32, kind="ExternalInput")
    bvb_d = nc.dram_tensor("bvb", [128, H * E], FP16, kind="ExternalInput")
    bpb_d = nc.dram_tensor("bpb", [128, H * E], F32, kind="ExternalInput")
    maskt_d = nc.dram_tensor("maskt", [128, 256], FP16, kind="ExternalInput")
    out_d = nc.dram_tensor("out", [T, D], F32, kind="ExternalOutput")

    Exp = mybir.ActivationFunctionType.Exp
    DR = mybir.MatmulPerfMode.DoubleRow

    with tile.TileContext(nc) as tc:
        with (
            tc.tile_pool(name="big", bufs=1) as big,
            tc.tile_pool(name="wf", bufs=2) as wf,
            tc.tile_pool(name="wb", bufs=4) as wb,
            tc.tile_pool(name="qk", bufs=4) as qk,
            tc.tile_pool(name="apool", bufs=8) as apool,
            tc.tile_pool(name="norm", bufs=3) as norm,
            tc.tile_pool(name="opool", bufs=3) as opool,
            tc.tile_pool(name="cst", bufs=1) as cst,
            tc.tile_pool(name="mps", bufs=2, space="PSUM") as mps,
            tc.tile_pool(name="sps", bufs=2, space="PSUM") as sps,
            tc.tile_pool(name="yps", bufs=2, space="PSUM") as yps,
        ):
            # ---- HAM warmup: junk matmuls with no DMA deps keep the PE busy
            # through its 3.4us activity window so real work runs at 2.4 GHz ----
            warm_t = cst.tile([128, 512], FP16)
            nc.gpsimd.memset(warm_t[:], 0.0)
            wps = mps.tile([128, 512], F32, tag="mps", name="warmps")
            for _ in range(16):
                nc.tensor.matmul(wps[:], warm_t[:, 0:128], warm_t[:],
                                 start=True, stop=True)

            # ---- constants ----
            bqk_t = cst.tile([128, 16], F32)
            bvb_t = cst.tile([128, H * E], FP16)
            bpb_t = cst.tile([128, H * E], F32)
            maskt_t = cst.tile([128, 256], FP16)
            onesf_t = cst.tile([128, 16], FP16)
            def _load_consts():
                nc.sync.dma_start(bqk_t[:], bqk_d.ap())
                nc.sync.dma_start(bvb_t[:], bvb_d.ap())
                nc.sync.dma_start(bpb_t[:], bpb_d.ap())
                nc.sync.dma_start(maskt_t[:], maskt_d.ap())
            nc.gpsimd.memset(onesf_t[:], 1.0)

            # ---- resident activations ----
            zT_t = [big.tile([128, T], FP16, tag="zTk", bufs=NK, name=f"zT{k}")
                    for k in range(NK)]
            x8_t = big.tile([128, NK, T], FP8, tag="x8")
            z8_t = big.tile([128, NK, T], FP8, tag="z8")
            V_t = big.tile([128, NJ, H * 65], FP16, tag="V")
            yT_t = big.tile([128, NP, T], FP16, tag="yT")
            wv_r = wv_d.ap().rearrange("(k p) he -> p k he", p=128)
            wp_r = wp_d.ap().rearrange("(k p) dout -> p k dout", p=128)
            wq_r = wq_d.ap().rearrange("(k p) he -> p k he", p=128)
            wk_r = wk_d.ap().rearrange("(k p) he -> p k he", p=128)
            x8_r = x8_d.ap().rearrange("(k p) t -> p k t", p=128)
            z8_r = z8_d.ap().rearrange("(k p) t -> p k t", p=128)
            zT_r = zT_d.ap().rearrange("(k p) t -> p k t", p=128)
            # zT + first wv half first, per-k tiles so the first V matmul only
            # waits on the k=0 chunks (dep tracking is tile-granular)
            wvh0 = [wf.tile([128, 512], FP16, tag="wv0", bufs=NK, name=f"wvh0_{k}")
                    for k in range(NK)]
            for k in range(NK):
                nc.sync.dma_start(zT_t[k][:], zT_r[:, k, :])
                nc.sync.dma_start(wvh0[k][:], wv_r[:, k, 0:512])
            _load_consts()
            nc.sync.dma_start(z8_t[:], z8_r[:, :, :])
            nc.sync.dma_start(x8_t[:], x8_r[:, :, :])

            # ---- V phase: V[z, he] = zT.T @ Wv + bv (65-col/head layout) ----
            for vc in range(2):
                wvh = None
                if vc == 1:
                    wvh = wf.tile([128, NK, 512], FP16, tag="wf")
                    nc.sync.dma_start(wvh[:], wv_r[:, :, 512:1024])
                for zb in range(NJ):
                    ps = mps.tile([128, 512], F32, tag="mps")
                    for k in range(NK):
                        if vc == 0 and zb == 0:
                            # dep-free filler keeps the PE busy (and the HAM
                            # clock-gate warm) while zT[k] chunks stream in
                            for _ in range(3):
                                nc.tensor.matmul(wps[:, 0:256], warm_t[:, 0:128],
                                                 warm_t[:, 0:256],
                                                 start=True, stop=True)
                        rhs = wvh0[k][:] if vc == 0 else wvh[:, k, :]
                        nc.tensor.matmul(
                            ps[:], zT_t[k][:, zb * 128:(zb + 1) * 128], rhs,
                            start=(k == 0), stop=(k == NK - 1),
                        )
                    dst = V_t[:, zb, vc * 520:(vc + 1) * 520].rearrange(
                        "p (h c) -> p h c", c=65)[:, :, 0:64]
                    nc.vector.tensor_add(
                        dst, ps[:].rearrange("p (h c) -> p h c", c=64),
                        bvb_t[:, vc * 512:(vc + 1) * 512].rearrange("p (h c) -> p h c", c=64))
            for zb in range(NJ):
                ones_dst = V_t[:, zb, :].rearrange("p (h c) -> p h c", c=65)[:, :, 64:65]
                nc.vector.tensor_copy(ones_dst, onesf_t[:].rearrange("p (h c) -> p h c", c=1))

            # ---- prefetch both Wp halves (wf slots free up after V-phase use) ----
            wph = []
            for dc in range(2):
                w = wf.tile([128, NK, 512], FP16, tag="wf", name=f"wph{dc}")
                nc.sync.dma_start(w[:], wp_r[:, :, dc * 512:(dc + 1) * 512])
                wph.append(w)

            # ---- head-pair loop (software-pipelined) ----
            # Per pair P: the attention j-loop runs AV one step behind S, and
            # the NEXT pair's Q/K projection matmuls are drip-fed between S and
            # AV so the PE always has dependency-free work while the ScalarE
            # exp (the rate limiter of this phase) catches up.
            def fetch_qk_weights(Pn):
                wqP = wb.tile([128, NK, 128], FP8, tag="wb", name=f"wq{Pn}")
                nc.sync.dma_start(wqP[:], wq_r[:, :, Pn * 128:(Pn + 1) * 128])
                wkP = wb.tile([128, NK, 128], FP8, tag="wb", name=f"wk{Pn}")
                nc.sync.dma_start(wkP[:], wk_r[:, :, Pn * 128:(Pn + 1) * 128])
                return wqP, wkP

            def qk_proj_thunks(Pn, wqP, wkP):
                """Per-instruction emission thunks for pair Pn's Q/K proj."""
                QT = qk.tile([128, T], FP16, tag="qk", name=f"QT{Pn}")
                KT = qk.tile([128, T], FP16, tag="qk", name=f"KT{Pn}")
                thunks = []
                for wi, (wt, act, dst, bcol) in enumerate((
                    (wqP, x8_t, QT, Pn), (wkP, z8_t, KT, 8 + Pn),
                )):
                    for c in range(NC):
                        state = {}
                        def alloc(state=state, nm=f"qkps{Pn}_{wi}_{c}"):
                            state["ps"] = mps.tile([128, 512], F32, tag="mps",
                                                   name=nm)
                        def mm(kp, wt=wt, act=act, c=c, state=state):
                            nc.tensor.matmul(
                                state["ps"][:], wt[:, 2 * kp:2 * kp + 2, :],
                                act[:, 2 * kp:2 * kp + 2, c * 512:(c + 1) * 512],
                                start=(kp == 0), stop=(kp == NKP - 1),
                                perf_mode=DR, skip_group_check=True,
                            )
                        def evict(dst=dst, c=c, bcol=bcol, state=state):
                            nc.vector.tensor_scalar_add(
                                dst[:, c * 512:(c + 1) * 512], state["ps"][:],
                                bqk_t[:, bcol:bcol + 1])
                        thunks.append(alloc)
                        for kp in range(NKP):
                            thunks.append(lambda kp=kp, mm=mm: mm(kp))
                        thunks.append(evict)
                return QT, KT, thunks

            # pair 0's projection runs un-pipelined (it follows the V phase)
            wq0, wk0 = fetch_qk_weights(0)
            QT, KT, th0 = qk_proj_thunks(0, wq0, wk0)
            for t in th0:
                t()

            for P in range(NP):
                # stage next pair's weights + thunk list
                fillers = []
                if P + 1 < NP:
                    wqN, wkN = fetch_qk_weights(P + 1)
                    QTn, KTn, fillers = qk_proj_thunks(P + 1, wqN, wkN)
                fill_i = [0]
                def pop_fill(n):
                    for _ in range(n):
                        if fill_i[0] < len(fillers):
                            fillers[fill_i[0]]()
                            fill_i[0] += 1

                seq = [(c, j) for c in range(NC)
                       for j in range(NJ) if 128 * j <= 512 * c + 511]
                last_of_c = {c: max(j for cc, j in seq if cc == c)
                             for c in range(NC)}
                yp_t = {}
                pend = None  # (c, j, at) awaiting AV + possibly norm

                def emit_av_and_norm(c, j, at):
                    x0 = 128 * max(j - 4 * c, 0)
                    for h01 in range(2):
                        h = 2 * P + h01
                        nc.tensor.matmul(
                            yp_t[c][h01][:, x0:512],
                            V_t[:, j, h * 65:(h + 1) * 65],
                            at[:, h01 * 512 + x0:(h01 + 1) * 512],
                            start=(j == 0), stop=(j == last_of_c[c]),
                            skip_group_check=True,
                        )
                    if j == last_of_c[c]:
                        # normalization + eviction to packed pair layout
                        # (sumexp must round-trip through SBUF:
                        # reciprocal_approx_fast is a bitwise-seed op and
                        # PSUM's e10m23 bits are not fp32)
                        for h01 in range(2):
                            hoff = 64 * h01
                            se_t = norm.tile([1, 512], F32, tag="se")
                            nc.vector.tensor_copy(se_t[:], yp_t[c][h01][64:65, :])
                            r_t = norm.tile([1, 512], F32, tag="rt")
                            nc.vector.reciprocal_approx_fast(r_t[:], se_t[:])
                            bc_t = norm.tile([64, 512], F32, tag="bc")
                            nc.gpsimd.partition_broadcast(bc_t[:], r_t[:])
                            nc.vector.tensor_mul(
                                yT_t[hoff:hoff + 64, P, c * 512:(c + 1) * 512],
                                yp_t[c][h01][0:64, :], bc_t[:])

                for (c, j) in seq:
                    if c not in yp_t:
                        yp_t[c] = [yps.tile([65, 512], F32, tag="yps",
                                            name=f"yp{P}_{c}_{h01}")
                                   for h01 in range(2)]
                    kband = j - 4 * c
                    x0 = 128 * max(kband, 0)
                    sp = sps.tile([128, 1024], F32, tag="sps")
                    at = apool.tile([128, 1024], FP16, tag="at")
                    for h01 in range(2):
                        hoff = 64 * h01
                        nc.tensor.matmul(
                            sp[:, h01 * 512 + x0:(h01 + 1) * 512],
                            KT[hoff:hoff + 64, j * 128:(j + 1) * 128],
                            QT[hoff:hoff + 64, c * 512 + x0:(c + 1) * 512],
                            start=True, stop=True, skip_group_check=True,
                        )
                    # one exp over both heads' regions (strided 2-bank AP)
                    sp_v = sp[:].rearrange("p (h x) -> p h x", x=512)[:, :, x0:512]
                    at_v = at[:].rearrange("p (h x) -> p h x", x=512)[:, :, x0:512]
                    nc.scalar.activation(at_v, sp_v, Exp, bias=0.0, scale=SCALE)
                    if kband >= 0:
                        at_m = at[:].rearrange(
                            "p (h x) -> p h x", x=512)[:, :, x0:x0 + 128]
                        mk_m = maskt_t[:].rearrange("p (h x) -> p h x", x=128)
                        nc.vector.tensor_mul(at_m, at_m, mk_m)
                    pop_fill(2)
                    if pend is not None:
                        emit_av_and_norm(*pend)
                    pend = (c, j, at)
                emit_av_and_norm(*pend)
                pop_fill(len(fillers))
                if P + 1 < NP:
                    QT, KT = QTn, KTn

            # ---- output projection: out = yT_cat.T @ Wp + bp ----
            for dc in range(2):
                for m in range(NJ):
                    ps = mps.tile([128, 512], F32, tag="mps")
                    for ht in range(NP):
                        nc.tensor.matmul(
                            ps[:], yT_t[:, ht, m * 128:(m + 1) * 128], wph[dc][:, ht, :],
                            start=(ht == 0), stop=(ht == NP - 1),
                        )
                    o_t = opool.tile([128, 512], F32, tag="ot")
                    nc.vector.tensor_add(o_t[:], ps[:], bpb_t[:, dc * 512:(dc + 1) * 512])
                    nc.sync.dma_start(
                        out_d.ap()[m * 128:(m + 1) * 128, dc * 512:(dc + 1) * 512],
                        o_t[:])

    nc.compile()
    return nc


_CACHED_NC = None


def _get_program():
    global _CACHED_NC
    if _CACHED_NC is None:
        _CACHED_NC = build_program()
    return _CACHED_NC


def _prep_shared(Wq, bq, Wk, bk, Wv, bv, Wp, bp, mask):
    assert np.array_equal(
        np.asarray(mask), np.tril(np.ones((T, T), dtype=bool))
    ), "kernel specialized for causal (tril) mask"
    wq = np.ascontiguousarray(
        np.asarray(Wq, np.float32).transpose(1, 0, 2).reshape(D, H * E).astype(E4M3))
    wk = np.ascontiguousarray(
        np.asarray(Wk, np.float32).transpose(1, 0, 2).reshape(D, H * E).astype(E4M3))
    wv = np.ascontiguousarray(
        np.asarray(Wv, np.float32).transpose(1, 0, 2).reshape(D, H * E).astype(np.float16))
    wp = np.ascontiguousarray(np.asarray(Wp, np.float32).astype(np.float16))
    bq_c = np.asarray(bq, np.float32).reshape(-1)
    bk_c = np.asarray(bk, np.float32).reshape(-1)
    bqk = np.concatenate(
        [bq_c.reshape(8, 128).T, bk_c.reshape(8, 128).T], axis=1
    ).astype(np.float32)
    tri = np.triu(np.ones((128, 128), np.float16))  # allow z <= x
    maskt = np.concatenate([tri, tri], axis=1)      # [128, 256] for both heads
    bvb = np.ascontiguousarray(np.broadcast_to(
        np.asarray(bv, np.float32).reshape(1, -1), (128, H * E)).astype(np.float16))
    bpb = np.ascontiguousarray(np.broadcast_to(
        np.asarray(bp, np.float32).reshape(1, -1), (128, H * E)).astype(np.float32))
    return {
        "wq": wq, "wk": wk, "wv": wv, "wp": wp,
        "bqk": np.ascontiguousarray(bqk),
        "bvb": bvb, "bpb": bpb,
        "maskt": np.ascontiguousarray(maskt),
    }


def kernel(x, z, Wq, bq, Wk, bk, Wv, bv, Wp, bp, mask, _trace=False, _trace_kwargs=None):
    x = np.asarray(x, np.float32)
    z = np.asarray(z, np.float32)
    shared = _prep_shared(Wq, bq, Wk, bk, Wv, bv, Wp, bp, mask)
    in_maps = []
    for b in range(B):
        m = dict(shared)
        zt = np.ascontiguousarray(z[b].T)
        m["zT"] = zt.astype(np.float16)
        m["z8"] = zt.astype(E4M3)
        m["x8"] = np.ascontiguousarray(x[b].T).astype(E4M3)
        in_maps.append(m)
    nc = _get_program()
    res = run_bass_kernel_spmd(
        nc, in_maps, core_ids=list(range(B)),
        trace=_trace, **(_trace_kwargs or {}),
    )
    out = np.stack([r["out"] for r in res.results]).astype(np.float32)
    if _trace:
        kernel.last_results = res
    return out
